# revision 1
# baseline (speedup 1.0000x reference)
"""Contextual-attention kernel for Trainium2, 8 NeuronCores, SPMD.

Decomposition (validated against the jax reference in numpy):
  scores[l,p] = rn[l] * sum_kk V[kk,l] * Gbox[kk,p]      (matmul1, kk=9*128)
  E = exp(scores - max_l scores)                          (softmax numerator)
  Mz[p,:] = sum_l E[l,p] * [rn[l]*V_lkk[l,:1152], 1]      (matmul2, Z in last col)
  out = col2im(Mz[:, :1152]/Z) * m/9 + fg*(1-m)           (host)

Sharding: core c handles sample c//2, pixel half c%2 (2048 of 4096 pixels).
No collectives; host scatters inputs / gathers outputs.
"""
import sys
for _p in ('/opt/trn_rl_repo',):
    if _p not in sys.path:
        sys.path.insert(0, _p)

import numpy as np

import concourse.bass as bass
import concourse.mybir as mybir
import concourse.tile as tile
from concourse import bacc
from concourse.bass_isa import ReduceOp
from concourse.bass_utils import run_bass_kernel_spmd

EPS = 1e-7
C, H, W = 128, 64, 64
L = H * W                      # 4096
KK = 9 * C                     # 1152
NC_COUNT = 8
HALF = L // 2                  # 2048 pixels per core
NCHUNK = 4                     # p-chunks of 512 per core
CW = 512                       # chunk width (pixels)
LT = 32                        # l-tiles of 128
PT_PER_CORE = 16               # p-tiles of 128 per core
DT_MM = mybir.dt.float32  # exact; float32r needs producer-side rounding
F32 = mybir.dt.float32

_compiled = None


def _build_program():
    nc = bacc.Bacc("TRN2", target_bir_lowering=False, debug=False)
    vslab_d = nc.dram_tensor("vslab", [C, 3 * 66 * 64], F32, kind="ExternalInput").ap()
    rnt_d = nc.dram_tensor("rnt", [C, LT], F32, kind="ExternalInput").ap()
    gsh_d = nc.dram_tensor("gsh", [9, C, HALF], F32, kind="ExternalInput").ap()
    vlkk2_d = nc.dram_tensor("vlkk2", [LT, C, KK + 1], F32, kind="ExternalInput").ap()
    mout_d = nc.dram_tensor("mout", [PT_PER_CORE, C, KK + 1], F32,
                            kind="ExternalOutput").ap()
    ident_d = nc.dram_tensor("ident", [C, C], F32, kind="ExternalInput").ap()
    ones1_d = nc.dram_tensor("ones1", [1, C], F32, kind="ExternalInput").ap()

    with tile.TileContext(nc) as tc:
        with (
            tc.tile_pool(name="const", bufs=1) as cpool,
            tc.tile_pool(name="gpool", bufs=2) as gpool,
            tc.tile_pool(name="sspool", bufs=1) as sspool,
            tc.tile_pool(name="small", bufs=2) as small,
            tc.tile_pool(name="vbufs", bufs=4) as vpool,
            tc.tile_pool(name="mo", bufs=4) as mopool,
            tc.tile_pool(name="ps1", bufs=2, space="PSUM") as ps1,
            tc.tile_pool(name="psm", bufs=2, space="PSUM") as psm,
            tc.tile_pool(name="ps2", bufs=4, space="PSUM") as ps2,
        ):
            vs = cpool.tile([C, 3 * 66 * 64], F32)
            nc.sync.dma_start(out=vs[:], in_=vslab_d[:])
            rnt = cpool.tile([C, LT], F32)
            nc.sync.dma_start(out=rnt[:], in_=rnt_d[:])
            ident = cpool.tile([C, C], F32)
            nc.sync.dma_start(out=ident[:], in_=ident_d[:])
            ones1 = cpool.tile([1, C], F32)
            nc.sync.dma_start(out=ones1[:], in_=ones1_d[:])

            for ch in range(NCHUNK):
                # ---- load G chunk: [128, 9, 512]
                gt = gpool.tile([C, 9, CW], F32, tag="gt")
                for k in range(9):
                    nc.sync.dma_start(out=gt[:, k, :],
                                      in_=gsh_d[k, :, ch * CW:(ch + 1) * CW])

                # ---- matmul1: ss[l, p] for all 32 l-tiles of this chunk
                ss = sspool.tile([C, LT * CW], F32, tag="ss")
                for lt in range(LT):
                    ps = ps1.tile([C, CW], F32, tag="ps1")
                    for k in range(9):
                        di, dj = k // 3, k % 3
                        base = (dj * 66 + 2 * lt + di) * 64
                        lhsT = vs[:, base:base + 128]
                        nc.tensor.matmul(ps[:], lhsT.bitcast(DT_MM),
                                         gt[:, k, :].bitcast(DT_MM),
                                         start=(k == 0), stop=(k == 8))
                    # drain with per-partition rn scale
                    nc.vector.tensor_scalar(
                        out=ss[:, lt * CW:(lt + 1) * CW], in0=ps[:],
                        scalar1=rnt[:, lt:lt + 1], scalar2=None,
                        op0=mybir.AluOpType.mult)

                # ---- max over l (32 tiles then across partitions)
                mrun = small.tile([C, CW], F32, tag="mrun")
                nc.vector.tensor_copy(out=mrun[:], in_=ss[:, 0:CW])
                for lt in range(1, LT):
                    nc.vector.tensor_tensor(out=mrun[:], in0=mrun[:],
                                            in1=ss[:, lt * CW:(lt + 1) * CW],
                                            op=mybir.AluOpType.max)
                # cross-partition max via PE: per 128-px block, transpose,
                # free-axis max, transpose back, ones-broadcast to all partitions
                mb = small.tile([C, CW], F32, tag="mb", name=f"mb_{ch}")
                for b in range(4):
                    tps = psm.tile([C, C], F32, tag="tp", name=f"tp_{ch}_{b}")
                    nc.tensor.transpose(tps[:], mrun[:, b * C:(b + 1) * C], ident[:])
                    tms = small.tile([C, C], F32, tag="tms", name=f"tms_{ch}_{b}")
                    nc.vector.tensor_copy(out=tms[:], in_=tps[:])
                    mcol = small.tile([C, 1], F32, tag="mcol", name=f"mc_{ch}_{b}")
                    nc.vector.tensor_reduce(mcol[:], tms[:],
                                            axis=mybir.AxisListType.XYZW,
                                            op=mybir.AluOpType.max)
                    tp2 = psm.tile([1, C], F32, tag="tp", name=f"tp2_{ch}_{b}")
                    nc.tensor.transpose(tp2[:], mcol[:], ident[:])
                    mrow = small.tile([1, C], F32, tag="mrow", name=f"mr_{ch}_{b}")
                    nc.vector.tensor_copy(out=mrow[:], in_=tp2[:])
                    bps = psm.tile([C, C], F32, tag="tp", name=f"bp_{ch}_{b}")
                    nc.tensor.matmul(bps[:], ones1[:], mrow[:], start=True, stop=True)
                    nc.vector.tensor_copy(out=mb[:, b * C:(b + 1) * C], in_=bps[:])
                mrun = mb

                # ---- exp(ss - m)
                for lt in range(LT):
                    sl = ss[:, lt * CW:(lt + 1) * CW]
                    nc.vector.tensor_tensor(out=sl, in0=sl, in1=mrun[:],
                                            op=mybir.AluOpType.subtract)
                    nc.scalar.activation(sl, sl, mybir.ActivationFunctionType.Exp)

                # ---- matmul2: Mz[p, kk] = sum_l E[l,p] * vlkk2[l,kk]
                for (c0, c1) in ((0, 512), (512, 1024), (1024, KK + 1)):
                    cw = c1 - c0
                    pss = [ps2.tile([C, 512], F32, tag="ps2", name=f"ps2_{ch}_{c0}_{i}")
                           for i in range(4)]
                    for ls in range(LT):
                        vb = vpool.tile([C, 512], F32, tag="vb")
                        nc.sync.dma_start(out=vb[:, :cw], in_=vlkk2_d[ls, :, c0:c1])
                        for pt in range(4):
                            lhsT = ss[:, ls * CW + pt * 128: ls * CW + (pt + 1) * 128]
                            nc.tensor.matmul(pss[pt][:, :cw], lhsT.bitcast(DT_MM),
                                             vb[:, :cw].bitcast(DT_MM),
                                             start=(ls == 0), stop=(ls == LT - 1))
                    for pt in range(4):
                        mo = mopool.tile([C, 512], F32, tag="mo")
                        nc.vector.tensor_copy(out=mo[:, :cw], in_=pss[pt][:, :cw])
                        nc.sync.dma_start(out=mout_d[ch * 4 + pt, :, c0:c1],
                                          in_=mo[:, :cw])
    nc.compile()
    return nc


def _host_prep(fg, m):
    """Per-sample operand tensors. fg [C,H,W] f32, m [1,H,W] f32."""
    bg = fg * (1.0 - m)
    vslab = (np.pad(bg, ((0, 0), (1, 1), (1, 1))) + EPS).astype(np.float32)

    v_lkk = np.empty((L, KK + 1), np.float32)
    for di in range(3):
        for dj in range(3):
            v_lkk[:, (di * 3 + dj) * C:(di * 3 + dj + 1) * C] = \
                vslab[:, di:di + H, dj:dj + W].reshape(C, L).T
    v_lkk[:, KK] = 1.0

    norm2 = np.sum(v_lkk[:, :KK].astype(np.float64) ** 2, axis=1)
    rn = (1.0 / np.sqrt(norm2)).astype(np.float32)
    rnt = np.ascontiguousarray(rn.reshape(LT, C).T)          # [128, 32]

    v_lkk2 = v_lkk.copy()
    v_lkk2[:, :KK] *= rn[:, None]
    vlkk2 = np.ascontiguousarray(v_lkk2.reshape(LT, C, KK + 1))

    fgpad = np.pad(fg, ((0, 0), (1, 1), (1, 1)))
    G = np.empty((9, C, L), np.float32)
    for di in range(3):
        for dj in range(3):
            Z = np.zeros((C, H + 2, W + 2), np.float32)
            Z[:, 1:H + 1, 1:W + 1] = fgpad[:, di:di + H, dj:dj + W]
            B = sum(Z[:, a:a + H, b:b + W] for a in range(3) for b in range(3))
            G[di * 3 + dj] = B.reshape(C, L)
    return vslab, rnt, vlkk2, G


def _host_post(Mpatch, fg, m):
    """col2im + final combine for one sample. Mpatch [L, 1152]."""
    rec = np.zeros((C, H, W), np.float32)
    Mp = Mpatch.reshape(H, W, 9, C)
    for di in range(3):
        for dj in range(3):
            oy, ox = 1 - di, 1 - dj
            ys, ye = max(0, -oy), min(H, H - oy)
            xs, xe = max(0, -ox), min(W, W - ox)
            rec[:, ys:ye, xs:xe] += np.transpose(
                Mp[ys + oy:ye + oy, xs + ox:xe + ox, di * 3 + dj, :], (2, 0, 1))
    return rec * m / 9.0 + fg * (1.0 - m)


def kernel(foreground, mask, _results_hook=None):
    global _compiled
    foreground = np.asarray(foreground, np.float32)
    mask = np.asarray(mask, np.float32)
    B = foreground.shape[0]

    if _compiled is None:
        _compiled = _build_program()
    nc = _compiled

    in_maps = []
    preps = []
    for s in range(B):
        vslab, rnt, vlkk2, G = _host_prep(foreground[s], mask[s])
        # [C,66,66] -> [C, 3(dj), 66, 64]: vs2[c,dj,y,x] = vslab[c,y,x+dj]
        vslab = np.ascontiguousarray(
            np.stack([vslab[:, :, dj:dj + 64] for dj in range(3)], axis=1)
        ).reshape(C, 3 * 66 * 64)
        preps.append((vslab, rnt, vlkk2, G))
    for core in range(NC_COUNT):
        s, h = core // 2, core % 2
        vslab, rnt, vlkk2, G = preps[s]
        in_maps.append({
            "vslab": vslab,
            "rnt": rnt,
            "gsh": np.ascontiguousarray(G[:, :, h * HALF:(h + 1) * HALF]),
            "vlkk2": vlkk2,
            "ident": np.eye(C, dtype=np.float32),
            "ones1": np.ones((1, C), np.float32),
        })

    res = run_bass_kernel_spmd(nc, in_maps, list(range(NC_COUNT)))
    if _results_hook is not None:
        _results_hook(res)

    out = np.empty_like(foreground)
    for s in range(B):
        halves = []
        for h in range(2):
            mo = np.asarray(res.results[2 * s + h]["mout"])      # [16,128,1153]
            halves.append(mo.transpose(0, 1, 2).reshape(HALF, KK + 1))
        Mz = np.concatenate(halves, axis=0)                       # [L, 1153]
        Mpatch = Mz[:, :KK] / Mz[:, KK:KK + 1]
        out[s] = _host_post(Mpatch, foreground[s], mask[s])
    return out



# revision 9
# speedup vs baseline: 39.6404x; 39.6404x over previous
"""Contextual-attention kernel for Trainium2, 8 NeuronCores, SPMD.

v2: full on-device pipeline. The axon tunnel moves ~50-70MB/s, so the
previous design (host-side prep, ~35MB in + 9.4MB out per core) was
entirely transfer-bound. Each core now receives only the full-sample
feature map fg [128,4096] + mask m [1,4096] (+ tiny constants) and runs
the complete per-sample pipeline on device:

  bg = fg*(1-m);  V = pad(bg)+EPS  (patch value slab)
  rn[l] = rsqrt(box3x3(sum_c V^2)[l])                 (patch norms)
  G[k]  = clipped-box3x3 of k-shifted 1-pad(fg)       (9 shifts)
  ss[l,p] = rn[l] * sum_{c,k} V[k,c,l] G[k][c,p]      (matmul1)
  E = exp(ss - max_l ss)
  Mz[p,:] = sum_l E[l,p] * [rn*V_l, 1]                (matmul2, Z last col)
  A = Mz[:,:1152]/Mz[:,1152];  rec = col2im(A)
  out_full = rec*m/9 + bg

Core 2s+h handles sample s; it outputs half h of out_full selected by
the 0/1 per-partition weights in wsel, so the SPMD program is identical
on every core (only input data differs).
"""
import sys
for _p in ('/opt/trn_rl_repo',):
    if _p not in sys.path:
        sys.path.insert(0, _p)

import numpy as np

import concourse.bass as bass
import concourse.mybir as mybir
import concourse.tile as tile
from concourse import bacc
from concourse.bass_utils import run_bass_kernel_spmd
from concourse import bass2jax

EPS = 1e-7
C, H, W = 128, 64, 64
L = H * W                      # 4096
KK = 9 * C                     # 1152
LT = 32                        # l-tiles of 128
CW = 256                       # pixel-chunk width
NCH = L // CW                  # 16 chunks
PT = CW // 128                 # p-tiles per chunk (2)
NC_COUNT = 8
F32 = mybir.dt.float32
AF = mybir.ActivationFunctionType
OP = mybir.AluOpType

_compiled = None


def _build_program():
    nc = bacc.Bacc("TRN2", target_bir_lowering=False, debug=False)
    fg_d = nc.dram_tensor("fg", [C, H, W], F32, kind="ExternalInput").ap()
    m_d = nc.dram_tensor("m", [1, L], F32, kind="ExternalInput").ap()
    ident_d = nc.dram_tensor("ident", [C, C], F32, kind="ExternalInput").ap()
    ones_d = nc.dram_tensor("ones", [C, C], F32, kind="ExternalInput").ap()
    wsel_d = nc.dram_tensor("wsel", [C, 2], F32, kind="ExternalInput").ap()
    out_d = nc.dram_tensor("out", [C, H // 2, W], F32, kind="ExternalOutput").ap()

    with tile.TileContext(nc) as tc:
        with (
            tc.tile_pool(name="keep", bufs=1) as keep,
            tc.tile_pool(name="dram", bufs=1, space="DRAM") as dpool,
            tc.tile_pool(name="psT", bufs=2, space="PSUM") as psT,
        ):
            ident = keep.tile([C, C], F32)
            nc.sync.dma_start(out=ident[:], in_=ident_d[:])
            ones = keep.tile([C, C], F32)
            nc.sync.dma_start(out=ones[:], in_=ones_d[:])
            wsel = keep.tile([C, 2], F32)
            nc.sync.dma_start(out=wsel[:], in_=wsel_d[:])

            bg3 = keep.tile([C, H, W], F32)       # fg*(1-m), resident
            mb3 = keep.tile([C, H, W], F32)       # mask bcast to partitions
            rnt = keep.tile([C, LT], F32)         # 1/|V_l| per l-tile column
            recpad = keep.tile([C, H + 2, W + 2], F32)
            nc.vector.memset(recpad[:], 0.0)

            G_dram = dpool.tile([9, C, L], F32)
            vlkk_dram = dpool.tile([LT, C, KK + 1], F32)

            # ---------------- stage A1: mask bcast, bg, G path ----------
            with tc.tile_pool(name="a1", bufs=1) as a1, \
                 tc.tile_pool(name="gst", bufs=2) as gst:
                fgt = a1.tile([C, H, W], F32)
                nc.sync.dma_start(out=fgt[:], in_=fg_d[:])
                mt = a1.tile([1, L], F32)
                nc.sync.dma_start(out=mt[:], in_=m_d[:])

                # broadcast m to all partitions via PE rank-1 product
                for b in range(L // 512):
                    pb = psT.tile([C, 512], F32, tag="psT", name=f"pb{b}")
                    nc.tensor.matmul(pb[:], ones[0:1, :], mt[:, b * 512:(b + 1) * 512],
                                     start=True, stop=True)
                    nc.vector.tensor_copy(
                        out=mb3[:, b * 8:(b + 1) * 8, :].rearrange("p a b -> p (a b)"),
                        in_=pb[:])

                # bg = fg - fg*m
                nc.vector.tensor_tensor(out=bg3[:], in0=fgt[:], in1=mb3[:], op=OP.mult)
                nc.vector.tensor_tensor(out=bg3[:], in0=fgt[:], in1=bg3[:], op=OP.subtract)

                # fg 1-pad
                fgp = a1.tile([C, H + 2, W + 2], F32)
                nc.vector.memset(fgp[:], 0.0)
                nc.vector.tensor_copy(out=fgp[:, 1:65, 1:65], in_=fgt[:])

                # column pass: Hs[dj][c,Y,pj] = sum_{x in [pj-1,pj+1] cap [0,63]} fgp[c,Y,x+dj]
                Hs = []
                for dj in range(3):
                    Ht = a1.tile([C, 66, 64], F32, name=f"H{dj}")
                    nc.vector.tensor_copy(out=Ht[:], in_=fgp[:, :, dj:dj + 64])
                    nc.vector.tensor_tensor(out=Ht[:, :, 1:64], in0=Ht[:, :, 1:64],
                                            in1=fgp[:, :, dj:dj + 63], op=OP.add)
                    nc.vector.tensor_tensor(out=Ht[:, :, 0:63], in0=Ht[:, :, 0:63],
                                            in1=fgp[:, :, dj + 1:dj + 64], op=OP.add)
                    Hs.append(Ht)

                # row pass + store G[k]
                for k in range(9):
                    di, dj = k // 3, k % 3
                    Gt = gst.tile([C, H, W], F32, tag="gstage", name=f"G{k}")
                    nc.vector.tensor_copy(out=Gt[:], in_=Hs[dj][:, di:di + 64, :])
                    nc.vector.tensor_tensor(out=Gt[:, 1:64, :], in0=Gt[:, 1:64, :],
                                            in1=Hs[dj][:, di:di + 63, :], op=OP.add)
                    nc.vector.tensor_tensor(out=Gt[:, 0:63, :], in0=Gt[:, 0:63, :],
                                            in1=Hs[dj][:, di + 1:di + 64, :], op=OP.add)
                    nc.sync.dma_start(out=G_dram[k], in_=Gt[:])

            # ---------------- stage A2: V slab, norms, vlkk2 ------------
            vsp_ctx = tc.tile_pool(name="vsp", bufs=1)
            vsp = vsp_ctx.__enter__()
            vs = vsp.tile([C, 3, 66, 64], F32)    # dj-shifted V slabs (mm1 lhsT)
            with tc.tile_pool(name="a2", bufs=1) as a2, \
                 tc.tile_pool(name="vkp", bufs=2) as vkp:
                vslab = a2.tile([C, 66, 66], F32)
                nc.vector.memset(vslab[:], EPS)
                nc.vector.tensor_scalar(out=vslab[:, 1:65, 1:65], in0=bg3[:],
                                        scalar1=EPS, scalar2=None, op0=OP.add)
                for dj in range(3):
                    nc.vector.tensor_copy(out=vs[:, dj, :, :],
                                          in_=vslab[:, :, dj:dj + 64])

                # norms: box3x3 of sum_c vslab^2, in l layout
                # (square vslab in place -- vs copies above already consumed it)
                sqs = vslab
                nc.scalar.activation(sqs[:], vslab[:], AF.Square)
                colT = a2.tile([C, 66, 64], F32)
                nc.vector.tensor_tensor(out=colT[:], in0=sqs[:, :, 0:64],
                                        in1=sqs[:, :, 1:65], op=OP.add)
                nc.vector.tensor_tensor(out=colT[:], in0=colT[:],
                                        in1=sqs[:, :, 2:66], op=OP.add)
                rowN = a2.tile([C, H, W], F32)
                nc.vector.tensor_tensor(out=rowN[:], in0=colT[:, 0:64, :],
                                        in1=colT[:, 1:65, :], op=OP.add)
                nc.vector.tensor_tensor(out=rowN[:], in0=rowN[:],
                                        in1=colT[:, 2:66, :], op=OP.add)
                nrow = a2.tile([1, L], F32)
                for b in range(L // 512):
                    pn = psT.tile([1, 512], F32, tag="psT", name=f"pn{b}")
                    nc.tensor.matmul(
                        pn[:], ones[:, 0:1],
                        rowN[:, b * 8:(b + 1) * 8, :].rearrange("p a b -> p (a b)"),
                        start=True, stop=True)
                    nc.vector.tensor_copy(out=nrow[:, b * 512:(b + 1) * 512], in_=pn[:])
                nc.vector.reciprocal(nrow[:], nrow[:])
                nc.scalar.activation(nrow[:], nrow[:], AF.Sqrt)
                for lt in range(LT):
                    pr = psT.tile([C, 1], F32, tag="psT", name=f"pr{lt}")
                    nc.tensor.transpose(pr[:], nrow[:, lt * C:(lt + 1) * C],
                                        ident[0:1, 0:1])
                    nc.vector.tensor_copy(out=rnt[:, lt:lt + 1], in_=pr[:])

                # vlkk2[lt] = [rn_l * V_l patches, 1] -> DRAM
                for lt in range(LT):
                    vt = vkp.tile([C, KK + 1], F32, tag="vt", name=f"vt{lt}")
                    for k in range(9):
                        di, dj = k // 3, k % 3
                        tp = psT.tile([C, C], F32, tag="psT", name=f"vtp{lt}_{k}")
                        nc.tensor.transpose(tp[:], vs[:, dj, 2 * lt + di:2 * lt + di + 2, :],
                                            ident[:])
                        nc.vector.tensor_scalar(out=vt[:, k * C:(k + 1) * C], in0=tp[:],
                                                scalar1=rnt[:, lt:lt + 1], scalar2=None,
                                                op0=OP.mult)
                    nc.vector.memset(vt[:, KK:KK + 1], 1.0)
                    nc.sync.dma_start(out=vlkk_dram[lt], in_=vt[:])

            # ---------------- chunk loop over pixels --------------------
            with (
                tc.tile_pool(name="ssp", bufs=1) as ssp,
                tc.tile_pool(name="gp", bufs=2) as gp,
                tc.tile_pool(name="vbp", bufs=4) as vbp,
                tc.tile_pool(name="mop", bufs=4) as mop,
                tc.tile_pool(name="sm", bufs=2) as sm,
                tc.tile_pool(name="ps1", bufs=2, space="PSUM") as ps1,
                tc.tile_pool(name="ps2", bufs=4, space="PSUM") as ps2,
            ):
                for ch in range(NCH):
                    gt = gp.tile([C, 9, CW], F32, tag="gt", name=f"gt{ch}")
                    for k in range(9):
                        nc.sync.dma_start(out=gt[:, k, :],
                                          in_=G_dram[k, :, ch * CW:(ch + 1) * CW])

                    # matmul1: ss[l, p] for all 32 l-tiles of this chunk
                    ss = ssp.tile([C, LT * CW], F32, tag="ss", name=f"ss{ch}")
                    for lt in range(LT):
                        ps = ps1.tile([C, CW], F32, tag="ps1", name=f"m1_{ch}_{lt}")
                        for k in range(9):
                            di, dj = k // 3, k % 3
                            nc.tensor.matmul(ps[:],
                                             vs[:, dj, 2 * lt + di:2 * lt + di + 2, :],
                                             gt[:, k, :],
                                             start=(k == 0), stop=(k == 8))
                        nc.scalar.activation(ss[:, lt * CW:(lt + 1) * CW], ps[:],
                                             AF.Copy, scale=rnt[:, lt:lt + 1])

                    # max over l: 31 tensor maxes then cross-partition
                    mrun = sm.tile([C, CW], F32, tag="mrun", name=f"mr{ch}")
                    nc.vector.tensor_copy(out=mrun[:], in_=ss[:, 0:CW])
                    for lt in range(1, LT):
                        nc.vector.tensor_tensor(out=mrun[:], in0=mrun[:],
                                                in1=ss[:, lt * CW:(lt + 1) * CW],
                                                op=OP.max)
                    mbx = sm.tile([C, CW], F32, tag="mbx", name=f"mb{ch}")
                    for b in range(PT):
                        tps = psT.tile([C, C], F32, tag="psT", name=f"tp{ch}_{b}")
                        nc.tensor.transpose(tps[:], mrun[:, b * C:(b + 1) * C], ident[:])
                        tms = sm.tile([C, C], F32, tag="tms", name=f"tm{ch}_{b}")
                        nc.vector.tensor_copy(out=tms[:], in_=tps[:])
                        mcol = sm.tile([C, 1], F32, tag="mcol", name=f"mc{ch}_{b}")
                        nc.vector.tensor_reduce(mcol[:], tms[:],
                                                axis=mybir.AxisListType.XYZW,
                                                op=OP.max)
                        tp2 = psT.tile([1, C], F32, tag="psT", name=f"t2{ch}_{b}")
                        nc.tensor.transpose(tp2[:], mcol[:], ident[:])
                        mrow = sm.tile([1, C], F32, tag="mrow", name=f"mw{ch}_{b}")
                        nc.vector.tensor_copy(out=mrow[:], in_=tp2[:])
                        bps = psT.tile([C, C], F32, tag="psT", name=f"bp{ch}_{b}")
                        nc.tensor.matmul(bps[:], ones[0:1, :], mrow[:],
                                         start=True, stop=True)
                        nc.vector.tensor_copy(out=mbx[:, b * C:(b + 1) * C], in_=bps[:])

                    # E = exp(ss - max)
                    for lt in range(LT):
                        sl = ss[:, lt * CW:(lt + 1) * CW]
                        nc.vector.tensor_tensor(out=sl, in0=sl, in1=mbx[:],
                                                op=OP.subtract)
                        nc.scalar.activation(sl, sl, AF.Exp)

                    # matmul2: Mz[p, kk] = sum_l E[l,p] vlkk2[l,kk]
                    mos = [mop.tile([C, KK + 1], F32, tag=f"mo{pt}", name=f"mo{ch}_{pt}")
                           for pt in range(PT)]
                    for (c0, c1) in ((0, 512), (512, 1024), (1024, KK + 1)):
                        cw = c1 - c0
                        pss = [ps2.tile([C, 512], F32, tag="ps2",
                                        name=f"p2_{ch}_{c0}_{i}") for i in range(PT)]
                        for lt in range(LT):
                            vb = vbp.tile([C, 512], F32, tag="vb",
                                          name=f"vb{ch}_{c0}_{lt}")
                            nc.sync.dma_start(out=vb[:, :cw],
                                              in_=vlkk_dram[lt, :, c0:c1])
                            for pt in range(PT):
                                lhsT = ss[:, lt * CW + pt * 128:lt * CW + (pt + 1) * 128]
                                nc.tensor.matmul(pss[pt][:, :cw], lhsT, vb[:, :cw],
                                                 start=(lt == 0), stop=(lt == LT - 1))
                        for pt in range(PT):
                            nc.vector.tensor_copy(out=mos[pt][:, c0:c1],
                                                  in_=pss[pt][:, :cw])

                    # A = Mz/Z, col2im scatter into recpad
                    for pt in range(PT):
                        g = ch * PT + pt          # global 2-row tile index
                        zr = sm.tile([C, 1], F32, tag="zr", name=f"zr{ch}_{pt}")
                        nc.vector.reciprocal(zr[:], mos[pt][:, KK:KK + 1])
                        nc.vector.tensor_scalar(out=mos[pt][:, 0:KK],
                                                in0=mos[pt][:, 0:KK],
                                                scalar1=zr[:], scalar2=None,
                                                op0=OP.mult)
                        for k in range(9):
                            di, dj = k // 3, k % 3
                            tpc = psT.tile([C, 2, 64], F32, tag="psT",
                                           name=f"c2_{ch}_{pt}_{k}")
                            nc.tensor.transpose(
                                tpc[:].rearrange("p a b -> p (a b)"),
                                mos[pt][:, k * C:(k + 1) * C], ident[:])
                            dst = recpad[:, 2 * g + di:2 * g + di + 2, dj:dj + 64]
                            nc.vector.tensor_tensor(out=dst, in0=dst, in1=tpc[:],
                                                    op=OP.add)

            vsp_ctx.__exit__(None, None, None)

            # ---------------- final combine + half select ---------------
            with tc.tile_pool(name="fin", bufs=1) as fin:
                of = fin.tile([C, H, W], F32)
                nc.vector.tensor_tensor(out=of[:], in0=recpad[:, 1:65, 1:65],
                                        in1=mb3[:], op=OP.mult)
                nc.vector.scalar_tensor_tensor(out=of[:], in0=of[:], scalar=1.0 / 9.0,
                                               in1=bg3[:], op0=OP.mult, op1=OP.add)
                osel = fin.tile([C, H // 2, W], F32)
                nc.vector.tensor_scalar(out=osel[:], in0=of[:, 0:32, :],
                                        scalar1=wsel[:, 0:1], scalar2=None,
                                        op0=OP.mult)
                nc.vector.scalar_tensor_tensor(out=osel[:], in0=of[:, 32:64, :],
                                               scalar=wsel[:, 1:2], in1=osel[:],
                                               op0=OP.mult, op1=OP.add)
                nc.sync.dma_start(out=out_d[:], in_=osel[:])
    nc.compile()
    return nc


_runner = None


def _make_runner(nc):
    """Build the same shard_map-jitted SPMD callable run_bass_kernel_spmd
    uses under axon (bass2jax.run_bass_via_pjrt), but construct it ONCE so
    repeat kernel() calls skip the ~1s client-side NEFF recompile the
    per-call jax.jit closure otherwise triggers."""
    import jax
    from jax.sharding import Mesh, PartitionSpec
    from jax.experimental.shard_map import shard_map

    bass2jax.install_neuronx_cc_hook()
    partition_name = (nc.partition_id_tensor.name
                      if nc.partition_id_tensor else None)
    in_names, out_names, out_avals, out_shapes = [], [], [], []
    for alloc in nc.m.functions[0].allocations:
        if not isinstance(alloc, mybir.MemoryLocationSet):
            continue
        name = alloc.memorylocations[0].name
        if alloc.kind == "ExternalInput":
            if name != partition_name:
                in_names.append(name)
        elif alloc.kind == "ExternalOutput":
            shape = tuple(alloc.tensor_shape)
            dtype = mybir.dt.np(alloc.dtype)
            out_avals.append(jax.core.ShapedArray(shape, dtype))
            out_names.append(name)
            out_shapes.append((shape, dtype))
    n_params = len(in_names)
    n_outs = len(out_names)
    all_in = list(in_names) + list(out_names)
    if partition_name is not None:
        all_in.append(partition_name)

    def _body(*args):
        operands = list(args)
        if partition_name is not None:
            operands.append(bass2jax.partition_id_tensor())
        outs = bass2jax._bass_exec_p.bind(
            *operands,
            out_avals=tuple(out_avals),
            in_names=tuple(all_in),
            out_names=tuple(out_names),
            lowering_input_output_aliases=(),
            sim_require_finite=True,
            sim_require_nnan=True,
            nc=nc,
        )
        return tuple(outs)

    devices = jax.devices()[:NC_COUNT]
    mesh = Mesh(np.asarray(devices), ("core",))
    donate = tuple(range(n_params, n_params + n_outs))
    sharded = jax.jit(
        shard_map(_body, mesh=mesh,
                  in_specs=(PartitionSpec("core"),) * (n_params + n_outs),
                  out_specs=(PartitionSpec("core"),) * n_outs,
                  check_rep=False),
        donate_argnums=donate, keep_unused=True)

    def run(in_maps):
        concat_in = [
            np.concatenate([np.asarray(in_maps[c][nm]) for c in range(NC_COUNT)],
                           axis=0)
            for nm in in_names
        ]
        concat_zeros = [np.zeros((NC_COUNT * s[0], *s[1:]), d)
                        for (s, d) in out_shapes]
        out_arrs = sharded(*concat_in, *concat_zeros)
        return [
            {nm: np.asarray(out_arrs[i]).reshape(NC_COUNT, *out_shapes[i][0])[c]
             for i, nm in enumerate(out_names)}
            for c in range(NC_COUNT)
        ]

    return run


def kernel(foreground, mask, _results_hook=None):
    global _compiled, _runner
    fg = np.asarray(foreground, np.float32)
    m = np.asarray(mask, np.float32)
    B = fg.shape[0]

    if _compiled is None:
        _compiled = _build_program()
    nc = _compiled

    ident = np.eye(C, dtype=np.float32)
    ones = np.ones((C, C), np.float32)
    in_maps = []
    for core in range(NC_COUNT):
        s, h = core // 2, core % 2
        wsel = np.zeros((C, 2), np.float32)
        wsel[:, h] = 1.0
        in_maps.append({
            "fg": np.ascontiguousarray(fg[s]),
            "m": np.ascontiguousarray(m[s].reshape(1, L)),
            "ident": ident,
            "ones": ones,
            "wsel": wsel,
        })

    try:
        if _runner is None:
            _runner = _make_runner(nc)
        results = _runner(in_maps)
    except Exception:
        def _fallback(ims):
            return run_bass_kernel_spmd(nc, ims, list(range(NC_COUNT))).results
        _runner = _fallback
        results = _runner(in_maps)

    out = np.empty_like(fg)
    for s in range(B):
        for h in range(2):
            half = np.asarray(results[2 * s + h]["out"])   # [C, 32, 64]
            out[s, :, h * 32:(h + 1) * 32, :] = half
    return out


# revision 17
# speedup vs baseline: 47.8959x; 1.2083x over previous
"""Contextual-attention kernel for Trainium2, 8 NeuronCores, SPMD.

v2: full on-device pipeline. The axon tunnel moves ~50-70MB/s, so the
previous design (host-side prep, ~35MB in + 9.4MB out per core) was
entirely transfer-bound. Each core now receives only the full-sample
feature map fg [128,4096] + mask m [1,4096] (+ tiny constants) and runs
the complete per-sample pipeline on device:

  bg = fg*(1-m);  V = pad(bg)+EPS  (patch value slab)
  rn[l] = rsqrt(box3x3(sum_c V^2)[l])                 (patch norms)
  G[k]  = clipped-box3x3 of k-shifted 1-pad(fg)       (9 shifts)
  ss[l,p] = rn[l] * sum_{c,k} V[k,c,l] G[k][c,p]      (matmul1)
  E = exp(ss - max_l ss)
  Mz[p,:] = sum_l E[l,p] * [rn*V_l, 1]                (matmul2, Z last col)
  A = Mz[:,:1152]/Mz[:,1152];  rec = col2im(A)
  out_full = rec*m/9 + bg

Core 2s+h handles sample s; it outputs half h of out_full selected by
the 0/1 per-partition weights in wsel, so the SPMD program is identical
on every core (only input data differs).
"""
import sys
for _p in ('/opt/trn_rl_repo',):
    if _p not in sys.path:
        sys.path.insert(0, _p)

import numpy as np

import concourse.bass as bass
import concourse.mybir as mybir
import concourse.tile as tile
from concourse import bacc
from concourse.bass_utils import run_bass_kernel_spmd
from concourse import bass2jax

EPS = 1e-7
C, H, W = 128, 64, 64
L = H * W                      # 4096
KK = 9 * C                     # 1152
LT = 32                        # l-tiles of 128
CW = 256                       # pixel-chunk width
NCH = L // CW                  # 16 chunks
PT = CW // 128                 # p-tiles per chunk (2)
NC_COUNT = 8
F32 = mybir.dt.float32
F16 = mybir.dt.float16
AF = mybir.ActivationFunctionType
OP = mybir.AluOpType

_compiled = None


def _build_program():
    nc = bacc.Bacc("TRN2", target_bir_lowering=False, debug=False)
    fg_d = nc.dram_tensor("fg", [C, H, W], F16, kind="ExternalInput").ap()
    m_d = nc.dram_tensor("m", [1, L], F32, kind="ExternalInput").ap()
    ident_d = nc.dram_tensor("ident", [C, C], F32, kind="ExternalInput").ap()
    ones_d = nc.dram_tensor("ones", [C, C], F32, kind="ExternalInput").ap()
    wsel_d = nc.dram_tensor("wsel", [C, 2], F32, kind="ExternalInput").ap()
    out_d = nc.dram_tensor("out", [C, H // 2, W], F16, kind="ExternalOutput").ap()

    with tile.TileContext(nc) as tc:
        with (
            tc.tile_pool(name="keep", bufs=1) as keep,
            tc.tile_pool(name="dram", bufs=1, space="DRAM") as dpool,
            tc.tile_pool(name="psT", bufs=2, space="PSUM") as psT,
        ):
            ident = keep.tile([C, C], F32)
            nc.sync.dma_start(out=ident[:], in_=ident_d[:])
            ones = keep.tile([C, C], F32)
            nc.sync.dma_start(out=ones[:], in_=ones_d[:])
            wsel = keep.tile([C, 2], F32)
            nc.sync.dma_start(out=wsel[:], in_=wsel_d[:])

            bg3 = keep.tile([C, H, W], F32)       # fg*(1-m), resident
            mb3 = keep.tile([C, H, W], F32)       # mask bcast to partitions
            rnt = keep.tile([C, LT], F32)         # 1/|V_l| per l-tile column
            recpad = keep.tile([C, H + 2, W + 2], F32)
            nc.vector.memset(recpad[:], 0.0)

            G_dram = dpool.tile([9, C, L], F32)
            vlkk_dram = dpool.tile([LT, C, KK + 1], F32)

            # ---------------- stage A1: mask bcast, bg, G path ----------
            with tc.tile_pool(name="a1", bufs=1) as a1, \
                 tc.tile_pool(name="gst", bufs=2) as gst:
                fgt16 = a1.tile([C, H, W], F16)
                nc.sync.dma_start(out=fgt16[:], in_=fg_d[:])
                fgt = a1.tile([C, H, W], F32)
                nc.vector.tensor_copy(out=fgt[:], in_=fgt16[:])
                mt = a1.tile([1, L], F32)
                nc.sync.dma_start(out=mt[:], in_=m_d[:])

                # broadcast m to all partitions via PE rank-1 product
                for b in range(L // 512):
                    pb = psT.tile([C, 512], F32, tag="psT", name=f"pb{b}")
                    nc.tensor.matmul(pb[:], ones[0:1, :], mt[:, b * 512:(b + 1) * 512],
                                     start=True, stop=True)
                    nc.vector.tensor_copy(
                        out=mb3[:, b * 8:(b + 1) * 8, :].rearrange("p a b -> p (a b)"),
                        in_=pb[:])

                # bg = fg - fg*m
                nc.vector.tensor_tensor(out=bg3[:], in0=fgt[:], in1=mb3[:], op=OP.mult)
                nc.vector.tensor_tensor(out=bg3[:], in0=fgt[:], in1=bg3[:], op=OP.subtract)

                # fg 1-pad
                fgp = a1.tile([C, H + 2, W + 2], F32)
                nc.vector.memset(fgp[:], 0.0)
                nc.vector.tensor_copy(out=fgp[:, 1:65, 1:65], in_=fgt[:])

                # column pass: Hs[dj][c,Y,pj] = sum_{x in [pj-1,pj+1] cap [0,63]} fgp[c,Y,x+dj]
                Hs = []
                for dj in range(3):
                    Ht = a1.tile([C, 66, 64], F32, name=f"H{dj}")
                    nc.vector.tensor_copy(out=Ht[:], in_=fgp[:, :, dj:dj + 64])
                    nc.vector.tensor_tensor(out=Ht[:, :, 1:64], in0=Ht[:, :, 1:64],
                                            in1=fgp[:, :, dj:dj + 63], op=OP.add)
                    nc.vector.tensor_tensor(out=Ht[:, :, 0:63], in0=Ht[:, :, 0:63],
                                            in1=fgp[:, :, dj + 1:dj + 64], op=OP.add)
                    Hs.append(Ht)

                # row pass + store G[k]
                for k in range(9):
                    di, dj = k // 3, k % 3
                    Gt = gst.tile([C, H, W], F32, tag="gstage", name=f"G{k}")
                    nc.vector.tensor_copy(out=Gt[:], in_=Hs[dj][:, di:di + 64, :])
                    nc.vector.tensor_tensor(out=Gt[:, 1:64, :], in0=Gt[:, 1:64, :],
                                            in1=Hs[dj][:, di:di + 63, :], op=OP.add)
                    nc.vector.tensor_tensor(out=Gt[:, 0:63, :], in0=Gt[:, 0:63, :],
                                            in1=Hs[dj][:, di + 1:di + 64, :], op=OP.add)
                    nc.sync.dma_start(out=G_dram[k], in_=Gt[:])

            # ---------------- stage A2: V slab, norms, vlkk2 ------------
            vsp_ctx = tc.tile_pool(name="vsp", bufs=1)
            vsp = vsp_ctx.__enter__()
            vs = vsp.tile([C, 3, 66, 64], F32)    # dj-shifted V slabs (mm1 lhsT)
            with tc.tile_pool(name="a2", bufs=1) as a2, \
                 tc.tile_pool(name="vkp", bufs=2) as vkp:
                vslab = a2.tile([C, 66, 66], F32)
                nc.vector.memset(vslab[:], EPS)
                nc.vector.tensor_scalar(out=vslab[:, 1:65, 1:65], in0=bg3[:],
                                        scalar1=EPS, scalar2=None, op0=OP.add)
                for dj in range(3):
                    nc.vector.tensor_copy(out=vs[:, dj, :, :],
                                          in_=vslab[:, :, dj:dj + 64])

                # norms: box3x3 of sum_c vslab^2, in l layout
                # (square vslab in place -- vs copies above already consumed it)
                sqs = vslab
                nc.scalar.activation(sqs[:], vslab[:], AF.Square)
                colT = a2.tile([C, 66, 64], F32)
                nc.vector.tensor_tensor(out=colT[:], in0=sqs[:, :, 0:64],
                                        in1=sqs[:, :, 1:65], op=OP.add)
                nc.vector.tensor_tensor(out=colT[:], in0=colT[:],
                                        in1=sqs[:, :, 2:66], op=OP.add)
                rowN = a2.tile([C, H, W], F32)
                nc.vector.tensor_tensor(out=rowN[:], in0=colT[:, 0:64, :],
                                        in1=colT[:, 1:65, :], op=OP.add)
                nc.vector.tensor_tensor(out=rowN[:], in0=rowN[:],
                                        in1=colT[:, 2:66, :], op=OP.add)
                nrow = a2.tile([1, L], F32)
                for b in range(L // 512):
                    pn = psT.tile([1, 512], F32, tag="psT", name=f"pn{b}")
                    nc.tensor.matmul(
                        pn[:], ones[:, 0:1],
                        rowN[:, b * 8:(b + 1) * 8, :].rearrange("p a b -> p (a b)"),
                        start=True, stop=True)
                    nc.vector.tensor_copy(out=nrow[:, b * 512:(b + 1) * 512], in_=pn[:])
                nc.vector.reciprocal(nrow[:], nrow[:])
                nc.scalar.activation(nrow[:], nrow[:], AF.Sqrt)
                for lt in range(LT):
                    pr = psT.tile([C, 1], F32, tag="psT", name=f"pr{lt}")
                    nc.tensor.transpose(pr[:], nrow[:, lt * C:(lt + 1) * C],
                                        ident[0:1, 0:1])
                    nc.vector.tensor_copy(out=rnt[:, lt:lt + 1], in_=pr[:])

                # vlkk2[lt] = [rn_l * V_l patches, 1] -> DRAM
                for lt in range(LT):
                    vt = vkp.tile([C, KK + 1], F32, tag="vt", name=f"vt{lt}")
                    for k in range(9):
                        di, dj = k // 3, k % 3
                        tp = psT.tile([C, C], F32, tag="psT", name=f"vtp{lt}_{k}")
                        nc.tensor.transpose(tp[:], vs[:, dj, 2 * lt + di:2 * lt + di + 2, :],
                                            ident[:])
                        nc.vector.tensor_scalar(out=vt[:, k * C:(k + 1) * C], in0=tp[:],
                                                scalar1=rnt[:, lt:lt + 1], scalar2=None,
                                                op0=OP.mult)
                    nc.vector.memset(vt[:, KK:KK + 1], 1.0)
                    nc.sync.dma_start(out=vlkk_dram[lt], in_=vt[:])

            # ---------------- chunk loop over pixels --------------------
            with (
                tc.tile_pool(name="ssp", bufs=1) as ssp,
                tc.tile_pool(name="gp", bufs=2) as gp,
                tc.tile_pool(name="vbp", bufs=4) as vbp,
                tc.tile_pool(name="mop", bufs=4) as mop,
                tc.tile_pool(name="sm", bufs=2) as sm,
                tc.tile_pool(name="ps1", bufs=2, space="PSUM") as ps1,
                tc.tile_pool(name="ps2", bufs=4, space="PSUM") as ps2,
            ):
                for ch in range(NCH):
                    gt = gp.tile([C, 9, CW], F32, tag="gt", name=f"gt{ch}")
                    for k in range(9):
                        nc.sync.dma_start(out=gt[:, k, :],
                                          in_=G_dram[k, :, ch * CW:(ch + 1) * CW])

                    # matmul1: ss[l, p] for all 32 l-tiles of this chunk
                    ss = ssp.tile([C, LT * CW], F32, tag="ss", name=f"ss{ch}")
                    for lt in range(LT):
                        ps = ps1.tile([C, CW], F32, tag="ps1", name=f"m1_{ch}_{lt}")
                        for k in range(9):
                            di, dj = k // 3, k % 3
                            nc.tensor.matmul(ps[:],
                                             vs[:, dj, 2 * lt + di:2 * lt + di + 2, :],
                                             gt[:, k, :],
                                             start=(k == 0), stop=(k == 8))
                        nc.scalar.activation(ss[:, lt * CW:(lt + 1) * CW], ps[:],
                                             AF.Copy, scale=rnt[:, lt:lt + 1])

                    # max over l: 31 tensor maxes then cross-partition
                    mrun = sm.tile([C, CW], F32, tag="mrun", name=f"mr{ch}")
                    nc.vector.tensor_copy(out=mrun[:], in_=ss[:, 0:CW])
                    for lt in range(1, LT):
                        nc.vector.tensor_tensor(out=mrun[:], in0=mrun[:],
                                                in1=ss[:, lt * CW:(lt + 1) * CW],
                                                op=OP.max)
                    mbx = sm.tile([C, CW], F32, tag="mbx", name=f"mb{ch}")
                    for b in range(PT):
                        tps = psT.tile([C, C], F32, tag="psT", name=f"tp{ch}_{b}")
                        nc.tensor.transpose(tps[:], mrun[:, b * C:(b + 1) * C], ident[:])
                        tms = sm.tile([C, C], F32, tag="tms", name=f"tm{ch}_{b}")
                        nc.vector.tensor_copy(out=tms[:], in_=tps[:])
                        mcol = sm.tile([C, 1], F32, tag="mcol", name=f"mc{ch}_{b}")
                        nc.vector.tensor_reduce(mcol[:], tms[:],
                                                axis=mybir.AxisListType.XYZW,
                                                op=OP.max)
                        tp2 = psT.tile([1, C], F32, tag="psT", name=f"t2{ch}_{b}")
                        nc.tensor.transpose(tp2[:], mcol[:], ident[:])
                        mrow = sm.tile([1, C], F32, tag="mrow", name=f"mw{ch}_{b}")
                        nc.vector.tensor_copy(out=mrow[:], in_=tp2[:])
                        bps = psT.tile([C, C], F32, tag="psT", name=f"bp{ch}_{b}")
                        nc.tensor.matmul(bps[:], ones[0:1, :], mrow[:],
                                         start=True, stop=True)
                        nc.vector.tensor_copy(out=mbx[:, b * C:(b + 1) * C], in_=bps[:])

                    # E = exp(ss - max)
                    for lt in range(LT):
                        sl = ss[:, lt * CW:(lt + 1) * CW]
                        nc.vector.tensor_tensor(out=sl, in0=sl, in1=mbx[:],
                                                op=OP.subtract)
                        nc.scalar.activation(sl, sl, AF.Exp)

                    # matmul2: Mz[p, kk] = sum_l E[l,p] vlkk2[l,kk]
                    mos = [mop.tile([C, KK + 1], F32, tag=f"mo{pt}", name=f"mo{ch}_{pt}")
                           for pt in range(PT)]
                    for (c0, c1) in ((0, 512), (512, 1024), (1024, KK + 1)):
                        cw = c1 - c0
                        pss = [ps2.tile([C, 512], F32, tag="ps2",
                                        name=f"p2_{ch}_{c0}_{i}") for i in range(PT)]
                        for lt in range(LT):
                            vb = vbp.tile([C, 512], F32, tag="vb",
                                          name=f"vb{ch}_{c0}_{lt}")
                            nc.sync.dma_start(out=vb[:, :cw],
                                              in_=vlkk_dram[lt, :, c0:c1])
                            for pt in range(PT):
                                lhsT = ss[:, lt * CW + pt * 128:lt * CW + (pt + 1) * 128]
                                nc.tensor.matmul(pss[pt][:, :cw], lhsT, vb[:, :cw],
                                                 start=(lt == 0), stop=(lt == LT - 1))
                        for pt in range(PT):
                            nc.vector.tensor_copy(out=mos[pt][:, c0:c1],
                                                  in_=pss[pt][:, :cw])

                    # A = Mz/Z, col2im scatter into recpad
                    for pt in range(PT):
                        g = ch * PT + pt          # global 2-row tile index
                        zr = sm.tile([C, 1], F32, tag="zr", name=f"zr{ch}_{pt}")
                        nc.vector.reciprocal(zr[:], mos[pt][:, KK:KK + 1])
                        nc.vector.tensor_scalar(out=mos[pt][:, 0:KK],
                                                in0=mos[pt][:, 0:KK],
                                                scalar1=zr[:], scalar2=None,
                                                op0=OP.mult)
                        for k in range(9):
                            di, dj = k // 3, k % 3
                            tpc = psT.tile([C, 2, 64], F32, tag="psT",
                                           name=f"c2_{ch}_{pt}_{k}")
                            nc.tensor.transpose(
                                tpc[:].rearrange("p a b -> p (a b)"),
                                mos[pt][:, k * C:(k + 1) * C], ident[:])
                            dst = recpad[:, 2 * g + di:2 * g + di + 2, dj:dj + 64]
                            nc.vector.tensor_tensor(out=dst, in0=dst, in1=tpc[:],
                                                    op=OP.add)

            vsp_ctx.__exit__(None, None, None)

            # ---------------- final combine + half select ---------------
            with tc.tile_pool(name="fin", bufs=1) as fin:
                of = fin.tile([C, H, W], F32)
                nc.vector.tensor_tensor(out=of[:], in0=recpad[:, 1:65, 1:65],
                                        in1=mb3[:], op=OP.mult)
                nc.vector.scalar_tensor_tensor(out=of[:], in0=of[:], scalar=1.0 / 9.0,
                                               in1=bg3[:], op0=OP.mult, op1=OP.add)
                osel = fin.tile([C, H // 2, W], F16)
                nc.vector.tensor_scalar(out=osel[:], in0=of[:, 0:32, :],
                                        scalar1=wsel[:, 0:1], scalar2=None,
                                        op0=OP.mult)
                nc.vector.scalar_tensor_tensor(out=osel[:], in0=of[:, 32:64, :],
                                               scalar=wsel[:, 1:2], in1=osel[:],
                                               op0=OP.mult, op1=OP.add)
                nc.sync.dma_start(out=out_d[:], in_=osel[:])
    nc.compile()
    return nc


_runner = None


def _make_runner(nc):
    """Build the same shard_map-jitted SPMD callable run_bass_kernel_spmd
    uses under axon (bass2jax.run_bass_via_pjrt), but construct it ONCE so
    repeat kernel() calls skip the ~1s client-side NEFF recompile the
    per-call jax.jit closure otherwise triggers."""
    import jax
    from jax.sharding import Mesh, PartitionSpec
    from jax.experimental.shard_map import shard_map

    bass2jax.install_neuronx_cc_hook()
    partition_name = (nc.partition_id_tensor.name
                      if nc.partition_id_tensor else None)
    in_names, out_names, out_avals, out_shapes = [], [], [], []
    for alloc in nc.m.functions[0].allocations:
        if not isinstance(alloc, mybir.MemoryLocationSet):
            continue
        name = alloc.memorylocations[0].name
        if alloc.kind == "ExternalInput":
            if name != partition_name:
                in_names.append(name)
        elif alloc.kind == "ExternalOutput":
            shape = tuple(alloc.tensor_shape)
            dtype = mybir.dt.np(alloc.dtype)
            out_avals.append(jax.core.ShapedArray(shape, dtype))
            out_names.append(name)
            out_shapes.append((shape, dtype))
    n_params = len(in_names)
    n_outs = len(out_names)
    all_in = list(in_names) + list(out_names)
    if partition_name is not None:
        all_in.append(partition_name)

    def _body(*args):
        operands = list(args)
        if partition_name is not None:
            operands.append(bass2jax.partition_id_tensor())
        outs = bass2jax._bass_exec_p.bind(
            *operands,
            out_avals=tuple(out_avals),
            in_names=tuple(all_in),
            out_names=tuple(out_names),
            lowering_input_output_aliases=(),
            sim_require_finite=True,
            sim_require_nnan=True,
            nc=nc,
        )
        return tuple(outs)

    devices = jax.devices()[:NC_COUNT]
    mesh = Mesh(np.asarray(devices), ("core",))
    donate = tuple(range(n_params, n_params + n_outs))
    sharded = jax.jit(
        shard_map(_body, mesh=mesh,
                  in_specs=(PartitionSpec("core"),) * (n_params + n_outs),
                  out_specs=(PartitionSpec("core"),) * n_outs,
                  check_rep=False),
        donate_argnums=donate, keep_unused=True)

    import os as _os
    import time as _time
    _dbg = _os.environ.get("BASSK_TIME")

    def run(in_maps):
        t0 = _time.time()
        concat_in = [
            np.concatenate([np.asarray(in_maps[c][nm]) for c in range(NC_COUNT)],
                           axis=0)
            for nm in in_names
        ]
        concat_zeros = [np.zeros((NC_COUNT * s[0], *s[1:]), d)
                        for (s, d) in out_shapes]
        t1 = _time.time()
        out_arrs = sharded(*concat_in, *concat_zeros)
        t2 = _time.time()
        if _dbg:
            jax.block_until_ready(out_arrs)
        t3 = _time.time()
        res = [
            {nm: np.asarray(out_arrs[i]).reshape(NC_COUNT, *out_shapes[i][0])[c]
             for i, nm in enumerate(out_names)}
            for c in range(NC_COUNT)
        ]
        if _dbg:
            t4 = _time.time()
            print(f"[bassk] concat {t1-t0:.3f} dispatch {t2-t1:.3f} "
                  f"block {t3-t2:.3f} fetch {t4-t3:.3f}")
        return res

    return run


def kernel(foreground, mask, _results_hook=None):
    global _compiled, _runner
    fg = np.asarray(foreground, np.float32)
    m = np.asarray(mask, np.float32)
    B = fg.shape[0]

    if _compiled is None:
        _compiled = _build_program()
    nc = _compiled

    ident = np.eye(C, dtype=np.float32)
    ones = np.ones((C, C), np.float32)
    in_maps = []
    for core in range(NC_COUNT):
        s, h = core // 2, core % 2
        wsel = np.zeros((C, 2), np.float32)
        wsel[:, h] = 1.0
        in_maps.append({
            "fg": np.ascontiguousarray(fg[s].astype(np.float16)),
            "m": np.ascontiguousarray(m[s].reshape(1, L)),
            "ident": ident,
            "ones": ones,
            "wsel": wsel,
        })

    try:
        if _runner is None:
            _runner = _make_runner(nc)
        results = _runner(in_maps)
    except Exception:
        def _fallback(ims):
            return run_bass_kernel_spmd(nc, ims, list(range(NC_COUNT))).results
        _runner = _fallback
        results = _runner(in_maps)

    out = np.empty_like(fg)
    for s in range(B):
        for h in range(2):
            half = np.asarray(results[2 * s + h]["out"])   # [C, 32, 64] fp16
            out[s, :, h * 32:(h + 1) * 32, :] = half.astype(np.float32)
    return out


# revision 19
# speedup vs baseline: 62.7417x; 1.3100x over previous
"""Contextual-attention kernel for Trainium2, 8 NeuronCores, SPMD.

v2: full on-device pipeline. The axon tunnel moves ~50-70MB/s, so the
previous design (host-side prep, ~35MB in + 9.4MB out per core) was
entirely transfer-bound. Each core now receives only the full-sample
feature map fg [128,4096] + mask m [1,4096] (+ tiny constants) and runs
the complete per-sample pipeline on device:

  bg = fg*(1-m);  V = pad(bg)+EPS  (patch value slab)
  rn[l] = rsqrt(box3x3(sum_c V^2)[l])                 (patch norms)
  G[k]  = clipped-box3x3 of k-shifted 1-pad(fg)       (9 shifts)
  ss[l,p] = rn[l] * sum_{c,k} V[k,c,l] G[k][c,p]      (matmul1)
  E = exp(ss - max_l ss)
  Mz[p,:] = sum_l E[l,p] * [rn*V_l, 1]                (matmul2, Z last col)
  A = Mz[:,:1152]/Mz[:,1152];  rec = col2im(A)
  out_full = rec*m/9 + bg

Core 2s+h handles sample s; it outputs half h of out_full selected by
the 0/1 per-partition weights in wsel, so the SPMD program is identical
on every core (only input data differs).
"""
import sys
for _p in ('/opt/trn_rl_repo',):
    if _p not in sys.path:
        sys.path.insert(0, _p)

import numpy as np

import concourse.bass as bass
import concourse.mybir as mybir
import concourse.tile as tile
from concourse import bacc
from concourse.bass_utils import run_bass_kernel_spmd
from concourse import bass2jax

EPS = 1e-7
C, H, W = 128, 64, 64
L = H * W                      # 4096
KK = 9 * C                     # 1152
LT = 32                        # l-tiles of 128
CW = 256                       # pixel-chunk width
NCH = L // CW                  # 16 chunks
PT = CW // 128                 # p-tiles per chunk (2)
NC_COUNT = 8
F32 = mybir.dt.float32
F16 = mybir.dt.float16
AF = mybir.ActivationFunctionType
OP = mybir.AluOpType

_compiled = None


def _build_program():
    nc = bacc.Bacc("TRN2", target_bir_lowering=False, debug=False)
    fg_d = nc.dram_tensor("fg", [C, H, W], F16, kind="ExternalInput").ap()
    m_d = nc.dram_tensor("m", [1, L], F32, kind="ExternalInput").ap()
    ident_d = nc.dram_tensor("ident", [C, C], F32, kind="ExternalInput").ap()
    ones_d = nc.dram_tensor("ones", [C, C], F32, kind="ExternalInput").ap()
    wsel_d = nc.dram_tensor("wsel", [C, 2], F32, kind="ExternalInput").ap()
    out_d = nc.dram_tensor("out", [C, H // 2, W], F16, kind="ExternalOutput").ap()

    with tile.TileContext(nc) as tc:
        with (
            tc.tile_pool(name="keep", bufs=1) as keep,
            tc.tile_pool(name="dram", bufs=1, space="DRAM") as dpool,
            tc.tile_pool(name="psT", bufs=2, space="PSUM") as psT,
        ):
            ident = keep.tile([C, C], F32)
            nc.sync.dma_start(out=ident[:], in_=ident_d[:])
            ones = keep.tile([C, C], F32)
            nc.sync.dma_start(out=ones[:], in_=ones_d[:])
            wsel = keep.tile([C, 2], F32)
            nc.sync.dma_start(out=wsel[:], in_=wsel_d[:])

            bg3 = keep.tile([C, H, W], F32)       # fg*(1-m), resident
            mb3 = keep.tile([C, H, W], F32)       # mask bcast to partitions
            rnt = keep.tile([C, LT], F32)         # 1/|V_l| per l-tile column
            recpad = keep.tile([C, H + 2, W + 2], F32)
            nc.vector.memset(recpad[:], 0.0)

            G_dram = dpool.tile([9, C, L], F32)
            vlkk_dram = dpool.tile([LT, C, KK + 1], F32)

            # ---------------- stage A1: mask bcast, bg, G path ----------
            with tc.tile_pool(name="a1", bufs=1) as a1, \
                 tc.tile_pool(name="gst", bufs=2) as gst:
                fgt16 = a1.tile([C, H, W], F16)
                nc.sync.dma_start(out=fgt16[:], in_=fg_d[:])
                fgt = a1.tile([C, H, W], F32)
                nc.vector.tensor_copy(out=fgt[:], in_=fgt16[:])
                mt = a1.tile([1, L], F32)
                nc.sync.dma_start(out=mt[:], in_=m_d[:])

                # broadcast m to all partitions via PE rank-1 product
                for b in range(L // 512):
                    pb = psT.tile([C, 512], F32, tag="psT", name=f"pb{b}")
                    nc.tensor.matmul(pb[:], ones[0:1, :], mt[:, b * 512:(b + 1) * 512],
                                     start=True, stop=True)
                    nc.vector.tensor_copy(
                        out=mb3[:, b * 8:(b + 1) * 8, :].rearrange("p a b -> p (a b)"),
                        in_=pb[:])

                # bg = fg - fg*m
                nc.vector.tensor_tensor(out=bg3[:], in0=fgt[:], in1=mb3[:], op=OP.mult)
                nc.vector.tensor_tensor(out=bg3[:], in0=fgt[:], in1=bg3[:], op=OP.subtract)

                # fg 1-pad
                fgp = a1.tile([C, H + 2, W + 2], F32)
                nc.vector.memset(fgp[:], 0.0)
                nc.vector.tensor_copy(out=fgp[:, 1:65, 1:65], in_=fgt[:])

                # column pass: Hs[dj][c,Y,pj] = sum_{x in [pj-1,pj+1] cap [0,63]} fgp[c,Y,x+dj]
                Hs = []
                for dj in range(3):
                    Ht = a1.tile([C, 66, 64], F32, name=f"H{dj}")
                    nc.vector.tensor_copy(out=Ht[:], in_=fgp[:, :, dj:dj + 64])
                    nc.vector.tensor_tensor(out=Ht[:, :, 1:64], in0=Ht[:, :, 1:64],
                                            in1=fgp[:, :, dj:dj + 63], op=OP.add)
                    nc.vector.tensor_tensor(out=Ht[:, :, 0:63], in0=Ht[:, :, 0:63],
                                            in1=fgp[:, :, dj + 1:dj + 64], op=OP.add)
                    Hs.append(Ht)

                # row pass + store G[k]
                for k in range(9):
                    di, dj = k // 3, k % 3
                    Gt = gst.tile([C, H, W], F32, tag="gstage", name=f"G{k}")
                    nc.vector.tensor_copy(out=Gt[:], in_=Hs[dj][:, di:di + 64, :])
                    nc.vector.tensor_tensor(out=Gt[:, 1:64, :], in0=Gt[:, 1:64, :],
                                            in1=Hs[dj][:, di:di + 63, :], op=OP.add)
                    nc.vector.tensor_tensor(out=Gt[:, 0:63, :], in0=Gt[:, 0:63, :],
                                            in1=Hs[dj][:, di + 1:di + 64, :], op=OP.add)
                    nc.sync.dma_start(out=G_dram[k], in_=Gt[:])

            # ---------------- stage A2: V slab, norms, vlkk2 ------------
            vsp_ctx = tc.tile_pool(name="vsp", bufs=1)
            vsp = vsp_ctx.__enter__()
            vs = vsp.tile([C, 3, 66, 64], F32)    # dj-shifted V slabs (mm1 lhsT)
            with tc.tile_pool(name="a2", bufs=1) as a2, \
                 tc.tile_pool(name="vkp", bufs=2) as vkp:
                vslab = a2.tile([C, 66, 66], F32)
                nc.vector.memset(vslab[:], EPS)
                nc.vector.tensor_scalar(out=vslab[:, 1:65, 1:65], in0=bg3[:],
                                        scalar1=EPS, scalar2=None, op0=OP.add)
                for dj in range(3):
                    nc.vector.tensor_copy(out=vs[:, dj, :, :],
                                          in_=vslab[:, :, dj:dj + 64])

                # norms: box3x3 of sum_c vslab^2, in l layout
                # (square vslab in place -- vs copies above already consumed it)
                sqs = vslab
                nc.scalar.activation(sqs[:], vslab[:], AF.Square)
                colT = a2.tile([C, 66, 64], F32)
                nc.vector.tensor_tensor(out=colT[:], in0=sqs[:, :, 0:64],
                                        in1=sqs[:, :, 1:65], op=OP.add)
                nc.vector.tensor_tensor(out=colT[:], in0=colT[:],
                                        in1=sqs[:, :, 2:66], op=OP.add)
                rowN = a2.tile([C, H, W], F32)
                nc.vector.tensor_tensor(out=rowN[:], in0=colT[:, 0:64, :],
                                        in1=colT[:, 1:65, :], op=OP.add)
                nc.vector.tensor_tensor(out=rowN[:], in0=rowN[:],
                                        in1=colT[:, 2:66, :], op=OP.add)
                nrow = a2.tile([1, L], F32)
                for b in range(L // 512):
                    pn = psT.tile([1, 512], F32, tag="psT", name=f"pn{b}")
                    nc.tensor.matmul(
                        pn[:], ones[:, 0:1],
                        rowN[:, b * 8:(b + 1) * 8, :].rearrange("p a b -> p (a b)"),
                        start=True, stop=True)
                    nc.vector.tensor_copy(out=nrow[:, b * 512:(b + 1) * 512], in_=pn[:])
                nc.vector.reciprocal(nrow[:], nrow[:])
                nc.scalar.activation(nrow[:], nrow[:], AF.Sqrt)
                for lt in range(LT):
                    pr = psT.tile([C, 1], F32, tag="psT", name=f"pr{lt}")
                    nc.tensor.transpose(pr[:], nrow[:, lt * C:(lt + 1) * C],
                                        ident[0:1, 0:1])
                    nc.vector.tensor_copy(out=rnt[:, lt:lt + 1], in_=pr[:])

                # vlkk2[lt] = [rn_l * V_l patches, 1] -> DRAM
                for lt in range(LT):
                    vt = vkp.tile([C, KK + 1], F32, tag="vt", name=f"vt{lt}")
                    for k in range(9):
                        di, dj = k // 3, k % 3
                        tp = psT.tile([C, C], F32, tag="psT", name=f"vtp{lt}_{k}")
                        nc.tensor.transpose(tp[:], vs[:, dj, 2 * lt + di:2 * lt + di + 2, :],
                                            ident[:])
                        nc.vector.tensor_scalar(out=vt[:, k * C:(k + 1) * C], in0=tp[:],
                                                scalar1=rnt[:, lt:lt + 1], scalar2=None,
                                                op0=OP.mult)
                    nc.vector.memset(vt[:, KK:KK + 1], 1.0)
                    nc.sync.dma_start(out=vlkk_dram[lt], in_=vt[:])

            # ---------------- chunk loop over pixels --------------------
            with (
                tc.tile_pool(name="ssp", bufs=1) as ssp,
                tc.tile_pool(name="gp", bufs=2) as gp,
                tc.tile_pool(name="vbp", bufs=4) as vbp,
                tc.tile_pool(name="mop", bufs=4) as mop,
                tc.tile_pool(name="sm", bufs=2) as sm,
                tc.tile_pool(name="ps1", bufs=2, space="PSUM") as ps1,
                tc.tile_pool(name="ps2", bufs=4, space="PSUM") as ps2,
            ):
                for ch in range(NCH):
                    gt = gp.tile([C, 9, CW], F32, tag="gt", name=f"gt{ch}")
                    for k in range(9):
                        nc.sync.dma_start(out=gt[:, k, :],
                                          in_=G_dram[k, :, ch * CW:(ch + 1) * CW])

                    # matmul1: ss[l, p] for all 32 l-tiles of this chunk
                    ss = ssp.tile([C, LT * CW], F32, tag="ss", name=f"ss{ch}")
                    for lt in range(LT):
                        ps = ps1.tile([C, CW], F32, tag="ps1", name=f"m1_{ch}_{lt}")
                        for k in range(9):
                            di, dj = k // 3, k % 3
                            nc.tensor.matmul(ps[:],
                                             vs[:, dj, 2 * lt + di:2 * lt + di + 2, :],
                                             gt[:, k, :],
                                             start=(k == 0), stop=(k == 8))
                        nc.scalar.activation(ss[:, lt * CW:(lt + 1) * CW], ps[:],
                                             AF.Copy, scale=rnt[:, lt:lt + 1])

                    # max over l: 31 tensor maxes then cross-partition
                    mrun = sm.tile([C, CW], F32, tag="mrun", name=f"mr{ch}")
                    nc.vector.tensor_copy(out=mrun[:], in_=ss[:, 0:CW])
                    for lt in range(1, LT):
                        nc.vector.tensor_tensor(out=mrun[:], in0=mrun[:],
                                                in1=ss[:, lt * CW:(lt + 1) * CW],
                                                op=OP.max)
                    mbx = sm.tile([C, CW], F32, tag="mbx", name=f"mb{ch}")
                    for b in range(PT):
                        tps = psT.tile([C, C], F32, tag="psT", name=f"tp{ch}_{b}")
                        nc.tensor.transpose(tps[:], mrun[:, b * C:(b + 1) * C], ident[:])
                        tms = sm.tile([C, C], F32, tag="tms", name=f"tm{ch}_{b}")
                        nc.vector.tensor_copy(out=tms[:], in_=tps[:])
                        mcol = sm.tile([C, 1], F32, tag="mcol", name=f"mc{ch}_{b}")
                        nc.vector.tensor_reduce(mcol[:], tms[:],
                                                axis=mybir.AxisListType.XYZW,
                                                op=OP.max)
                        tp2 = psT.tile([1, C], F32, tag="psT", name=f"t2{ch}_{b}")
                        nc.tensor.transpose(tp2[:], mcol[:], ident[:])
                        mrow = sm.tile([1, C], F32, tag="mrow", name=f"mw{ch}_{b}")
                        nc.vector.tensor_copy(out=mrow[:], in_=tp2[:])
                        bps = psT.tile([C, C], F32, tag="psT", name=f"bp{ch}_{b}")
                        nc.tensor.matmul(bps[:], ones[0:1, :], mrow[:],
                                         start=True, stop=True)
                        nc.vector.tensor_copy(out=mbx[:, b * C:(b + 1) * C], in_=bps[:])

                    # E = exp(ss - max)
                    for lt in range(LT):
                        sl = ss[:, lt * CW:(lt + 1) * CW]
                        nc.vector.tensor_tensor(out=sl, in0=sl, in1=mbx[:],
                                                op=OP.subtract)
                        nc.scalar.activation(sl, sl, AF.Exp)

                    # matmul2: Mz[p, kk] = sum_l E[l,p] vlkk2[l,kk]
                    mos = [mop.tile([C, KK + 1], F32, tag=f"mo{pt}", name=f"mo{ch}_{pt}")
                           for pt in range(PT)]
                    for (c0, c1) in ((0, 512), (512, 1024), (1024, KK + 1)):
                        cw = c1 - c0
                        pss = [ps2.tile([C, 512], F32, tag="ps2",
                                        name=f"p2_{ch}_{c0}_{i}") for i in range(PT)]
                        for lt in range(LT):
                            vb = vbp.tile([C, 512], F32, tag="vb",
                                          name=f"vb{ch}_{c0}_{lt}")
                            nc.sync.dma_start(out=vb[:, :cw],
                                              in_=vlkk_dram[lt, :, c0:c1])
                            for pt in range(PT):
                                lhsT = ss[:, lt * CW + pt * 128:lt * CW + (pt + 1) * 128]
                                nc.tensor.matmul(pss[pt][:, :cw], lhsT, vb[:, :cw],
                                                 start=(lt == 0), stop=(lt == LT - 1))
                        for pt in range(PT):
                            nc.vector.tensor_copy(out=mos[pt][:, c0:c1],
                                                  in_=pss[pt][:, :cw])

                    # A = Mz/Z, col2im scatter into recpad
                    for pt in range(PT):
                        g = ch * PT + pt          # global 2-row tile index
                        zr = sm.tile([C, 1], F32, tag="zr", name=f"zr{ch}_{pt}")
                        nc.vector.reciprocal(zr[:], mos[pt][:, KK:KK + 1])
                        nc.vector.tensor_scalar(out=mos[pt][:, 0:KK],
                                                in0=mos[pt][:, 0:KK],
                                                scalar1=zr[:], scalar2=None,
                                                op0=OP.mult)
                        for k in range(9):
                            di, dj = k // 3, k % 3
                            tpc = psT.tile([C, 2, 64], F32, tag="psT",
                                           name=f"c2_{ch}_{pt}_{k}")
                            nc.tensor.transpose(
                                tpc[:].rearrange("p a b -> p (a b)"),
                                mos[pt][:, k * C:(k + 1) * C], ident[:])
                            dst = recpad[:, 2 * g + di:2 * g + di + 2, dj:dj + 64]
                            nc.vector.tensor_tensor(out=dst, in0=dst, in1=tpc[:],
                                                    op=OP.add)

            vsp_ctx.__exit__(None, None, None)

            # ---------------- final combine + half select ---------------
            with tc.tile_pool(name="fin", bufs=1) as fin:
                of = fin.tile([C, H, W], F32)
                nc.vector.tensor_tensor(out=of[:], in0=recpad[:, 1:65, 1:65],
                                        in1=mb3[:], op=OP.mult)
                nc.vector.scalar_tensor_tensor(out=of[:], in0=of[:], scalar=1.0 / 9.0,
                                               in1=bg3[:], op0=OP.mult, op1=OP.add)
                osel = fin.tile([C, H // 2, W], F16)
                nc.vector.tensor_scalar(out=osel[:], in0=of[:, 0:32, :],
                                        scalar1=wsel[:, 0:1], scalar2=None,
                                        op0=OP.mult)
                nc.vector.scalar_tensor_tensor(out=osel[:], in0=of[:, 32:64, :],
                                               scalar=wsel[:, 1:2], in1=osel[:],
                                               op0=OP.mult, op1=OP.add)
                nc.sync.dma_start(out=out_d[:], in_=osel[:])
    nc.compile()
    return nc


_runner = None


def _make_runner(nc):
    """Build the same shard_map-jitted SPMD callable run_bass_kernel_spmd
    uses under axon (bass2jax.run_bass_via_pjrt), but construct it ONCE so
    repeat kernel() calls skip the ~1s client-side NEFF recompile the
    per-call jax.jit closure otherwise triggers."""
    import jax
    from jax.sharding import Mesh, PartitionSpec
    from jax.experimental.shard_map import shard_map

    bass2jax.install_neuronx_cc_hook()
    partition_name = (nc.partition_id_tensor.name
                      if nc.partition_id_tensor else None)
    in_names, out_names, out_avals, out_shapes = [], [], [], []
    for alloc in nc.m.functions[0].allocations:
        if not isinstance(alloc, mybir.MemoryLocationSet):
            continue
        name = alloc.memorylocations[0].name
        if alloc.kind == "ExternalInput":
            if name != partition_name:
                in_names.append(name)
        elif alloc.kind == "ExternalOutput":
            shape = tuple(alloc.tensor_shape)
            dtype = mybir.dt.np(alloc.dtype)
            out_avals.append(jax.core.ShapedArray(shape, dtype))
            out_names.append(name)
            out_shapes.append((shape, dtype))
    n_params = len(in_names)
    n_outs = len(out_names)
    all_in = list(in_names) + list(out_names)
    if partition_name is not None:
        all_in.append(partition_name)

    def _body(*args):
        operands = list(args)
        if partition_name is not None:
            operands.append(bass2jax.partition_id_tensor())
        outs = bass2jax._bass_exec_p.bind(
            *operands,
            out_avals=tuple(out_avals),
            in_names=tuple(all_in),
            out_names=tuple(out_names),
            lowering_input_output_aliases=(),
            sim_require_finite=True,
            sim_require_nnan=True,
            nc=nc,
        )
        return tuple(outs)

    devices = jax.devices()[:NC_COUNT]
    mesh = Mesh(np.asarray(devices), ("core",))
    donate = tuple(range(n_params, n_params + n_outs))
    sharded = jax.jit(
        shard_map(_body, mesh=mesh,
                  in_specs=(PartitionSpec("core"),) * (n_params + n_outs),
                  out_specs=(PartitionSpec("core"),) * n_outs,
                  check_rep=False),
        donate_argnums=donate, keep_unused=True)

    # donated zero output buffers, created ON DEVICE (skips a 4MB upload/call)
    import jax.numpy as jnp
    from jax.sharding import NamedSharding
    sh = NamedSharding(mesh, PartitionSpec("core"))
    zeros_jit = jax.jit(
        lambda: tuple(jnp.zeros((NC_COUNT * s[0], *s[1:]), d)
                      for (s, d) in out_shapes),
        out_shardings=(sh,) * n_outs)

    # inputs that never change across calls stay resident on device
    _static = {"ident", "ones", "wsel"}
    static_dev = {}

    def _stage(nm, concat):
        if nm in _static:
            if nm not in static_dev:
                static_dev[nm] = jax.device_put(concat, sh)
            return static_dev[nm]
        return concat

    import os as _os
    import time as _time
    _dbg = _os.environ.get("BASSK_TIME")

    def run(in_maps):
        t0 = _time.time()
        concat_in = [
            _stage(nm, np.concatenate([np.asarray(in_maps[c][nm])
                                       for c in range(NC_COUNT)], axis=0))
            for nm in in_names
        ]
        concat_zeros = zeros_jit()
        t1 = _time.time()
        out_arrs = sharded(*concat_in, *concat_zeros)
        t2 = _time.time()
        if _dbg:
            jax.block_until_ready(out_arrs)
        t3 = _time.time()
        res = [
            {nm: np.asarray(out_arrs[i]).reshape(NC_COUNT, *out_shapes[i][0])[c]
             for i, nm in enumerate(out_names)}
            for c in range(NC_COUNT)
        ]
        if _dbg:
            t4 = _time.time()
            print(f"[bassk] concat {t1-t0:.3f} dispatch {t2-t1:.3f} "
                  f"block {t3-t2:.3f} fetch {t4-t3:.3f}")
        return res

    return run


def kernel(foreground, mask, _results_hook=None):
    global _compiled, _runner
    fg = np.asarray(foreground, np.float32)
    m = np.asarray(mask, np.float32)
    B = fg.shape[0]

    if _compiled is None:
        _compiled = _build_program()
    nc = _compiled

    ident = np.eye(C, dtype=np.float32)
    ones = np.ones((C, C), np.float32)
    in_maps = []
    for core in range(NC_COUNT):
        s, h = core // 2, core % 2
        wsel = np.zeros((C, 2), np.float32)
        wsel[:, h] = 1.0
        in_maps.append({
            "fg": np.ascontiguousarray(fg[s].astype(np.float16)),
            "m": np.ascontiguousarray(m[s].reshape(1, L)),
            "ident": ident,
            "ones": ones,
            "wsel": wsel,
        })

    try:
        if _runner is None:
            _runner = _make_runner(nc)
        results = _runner(in_maps)
    except Exception:
        def _fallback(ims):
            return run_bass_kernel_spmd(nc, ims, list(range(NC_COUNT))).results
        _runner = _fallback
        results = _runner(in_maps)

    out = np.empty_like(fg)
    for s in range(B):
        for h in range(2):
            half = np.asarray(results[2 * s + h]["out"])   # [C, 32, 64] fp16
            out[s, :, h * 32:(h + 1) * 32, :] = half.astype(np.float32)
    return out
